# revision 81
# baseline (speedup 1.0000x reference)
"""Trainium2 Bass kernel: boson-sampler probabilities via Glynn's permanent.

Glynn with 2^17 terms on a [P=128, F=1024] grid; rows grouped 6+6+6 into
three rank-64 tables contracted against the grid (stage 4), then the
T0*T1*T2 signed reduce. Unified-width tiles [rows, 1152] carry the H (1024)
and G (128) sides through shared ops with per-512-chunk matmul constants.

Speed structure (vs the fp32 baseline):
- All selection/contraction matmuls run fp32r (1 cyc/col vs 4 for fp32).
  fp32r is ~10-bit-mantissa; anything consumed by an fp32r matmul must be
  *written* as fp32r (verifier rule), which rounds it. The final Glynn
  cancellation amplifies pre-reduction rounding ~100x, so:
  * the master row-sum stage is folded into per-input host lhsT columns
    (LTPACK blocks) multiplied against the exact +/-1 MASK tile - no
    on-device master tile, no rounding, and the whole RS stage disappears;
  * pair products use the ACT-Square trick (squares of host-folded sums),
    one rounded stage (sqt);
  * the quad layer is two raw product tiles m1/m2 kept in exact fp32, with
    the +/- combine folded into the REP* constants of plain-fp32 q-pack
    matmuls (4 cyc/col, paid for accuracy);
  * only the L2 tables (er/ei) remain rounded -> measured ~9e-3 rel err on
    HW vs the 2e-2 gate.
- Engine legality learned from walrus: GPSIMD touches SBUF only (plain
  tensor ops, no STT); at most one PSUM operand per DVE op; both-SBUF
  operands need equal base partitions; engine writes start at partition
  0/32/64/96. Products pair one PSUM with one SBUF operand where possible;
  Pool gets equal-base SBUF products fed by ACT evacuations.
- Stage 4 reads table tiles through views (G cols as lhsT, H cols as rhs)
  with two accumulating K=64 matmuls per chunk - no repack stage.
- Final: p01 = T0*T1 (DVE/Pool), four scalar_tensor_tensor accumulations
  against PSUM-resident T2, ones-matmul reduce, ACT-Square, one STT tail.
One NeuronCore per batch element; input DMAs split across the SP and ACT
queues (LTPACK in three pieces) so the ACT Square is never queue-blocked.
"""

import sys

sys.path.insert(0, "/opt/trn_rl_repo")

import numpy as np

import concourse.bacc as bacc
import concourse.bass as bass
import concourse.tile as tile
from concourse import mybir
from concourse.bass_utils import run_bass_kernel_spmd

FP32 = mybir.dt.float32
FP32R = mybir.dt.float32r
BF16 = mybir.dt.bfloat16
OP = mybir.AluOpType
AF = mybir.ActivationFunctionType

N = 18
PBITS, FBITS = 7, 10
P, F = 1 << PBITS, 1 << FBITS          # 128, 1024
W = F + P                              # 1152 unified width
EMU = 0.85 * (1 - 0.02) * (1 - 0.02) * (1 - 0.01)
DARK = 1e-6 * N
SCALE2 = float(2.0 ** (2 * (1 - N)))

# master rows (one tile): r-block base 0, i-block base 64
M_SE, M_SO, M_ONE, M_SGN = 18, 19, 20, 21
MI = 64                                # imag row offset
M_PP = 32                              # pair-product rows 32:42 (r), 96:106 (i)
M_ROWS = 96                            # rows written by the RS matmul
M_ROWS_PP = 106                        # master height incl. pair products
NQ = 10                                # pairs: 9 regular + 1 sign-scaled
SQROWS = 8 * NQ                        # 80
USE_SQ = True                          # pair products via ACT-square trick

CHUNKS = [(0, 512, "H"), (512, 1024, "H"), (1024, 1152, "G")]

_CACHE = {}


def _pm_mask(nvals, bits):
    v = np.arange(nvals, dtype=np.uint32)
    m = (v[:, None] >> np.arange(bits, dtype=np.uint32)[None, :]) & 1
    return (1.0 - 2.0 * m).astype(np.float32).T.copy()   # [bits, nvals]


def _parity(nvals, bits):
    v = np.arange(nvals, dtype=np.uint32)
    pc = np.zeros(nvals, dtype=np.uint32)
    for k in range(bits):
        pc += (v >> k) & 1
    return np.where(pc % 2 == 0, 1.0, -1.0).astype(np.float32)


# ---------------------------------------------------------------- linear forms
# A table entry is a pair (r_form, i_form); each form is a linear combination
# {"m": {master_row: coef}, "s": {sqp_row: coef}} over master / squares tiles.
PAIR_XY = [(2 * q, 2 * q + 1) for q in range(9)] + [(M_SE, 17)]


def _pp_form(q):
    b = 8 * q
    r = {"s": {b + 0: .25, b + 1: -.25, b + 2: -.25, b + 3: .25}}
    i = {"s": {b + 4: .25, b + 5: -.25, b + 6: .25, b + 7: -.25}}
    return (r, i)


def _row_form(k):
    return ({"m": {k: 1.0}}, {"m": {MI + k: 1.0}})


_ONES_F = ({"m": {M_ONE: 1.0}}, {})
_SGN_F = ({"m": {M_SGN: 1.0}}, {})


def tab_entry(side, q, z):
    """Pair-table entry (pair q, z in 0..3) as linear forms."""
    if side == "H":
        m = {0: "PP", 1: "odd", 2: "even", 3: "one"}[z]
    else:
        m = {0: "one", 1: "even", 2: "odd", 3: "PP"}[z]
    if m == "one":
        return _ONES_F
    if m == "PP":
        return _pp_form(q) if USE_SQ else _row_form(M_PP + q)
    k = 2 * q + (1 if m == "odd" else 0)
    return _row_form(k)


def tab2_entry(side, z):
    """Sign-scaled pair-8 table entry (uses scaled rows / scaled pair 9)."""
    if side == "H":
        m = {0: "PPS", 1: "SO", 2: "SE", 3: "sgn"}[z]
    else:
        m = {0: "sgn", 1: "SE", 2: "SO", 3: "PPS"}[z]
    if m == "sgn":
        return _SGN_F
    if m == "PPS":
        return _pp_form(9) if USE_SQ else _row_form(M_PP + 9)
    return _row_form(M_SE if m == "SE" else M_SO)


def _forms_to_mats(cols, nm_rows=M_ROWS_PP, ns_rows=SQROWS):
    """cols: list of forms -> (master-part [96, m], sqp-part [80, m])."""
    m = len(cols)
    selm = np.zeros((nm_rows, m), np.float32)
    sels = np.zeros((ns_rows, m), np.float32)
    for j, form in enumerate(cols):
        for k, c in form.get("m", {}).items():
            selm[k, j] += c
        for k, c in form.get("s", {}).items():
            sels[k, j] += c
    return selm, sels


# ---------------------------------------------------------------- constants
def _build_consts():
    C = {}
    # masks
    mF = np.concatenate([np.ones((1, F), np.float32), _pm_mask(F, FBITS)], 0)
    sF = _parity(F, FBITS)
    MFX = np.concatenate([mF, mF * sF[None, :]], 0)              # [22, F]
    mP = _pm_mask(P, PBITS)
    sP = _parity(P, PBITS)
    MPX = np.concatenate(
        [mP, mP * sP[None, :], np.ones((1, P), np.float32), sP[None, :]], 0)
    MASK = np.zeros((22, W), np.float32)
    MASK[0:22, 0:F] = MFX
    MASK[0:16, F:W] = MPX
    C["MASK"] = MASK

    # SELPSQ [96, 80]: square-input sum rows, 8 per pair
    # (s1..s8) = (xr+yr, xr-yr, xi+yi, xi-yi, xr+yi, xr-yi, xi+yr, xi-yr)
    sel = np.zeros((M_ROWS, SQROWS), np.float32)
    for q, (x, y) in enumerate(PAIR_XY):
        xr, xi, yr, yi = x, MI + x, y, MI + y
        for j, (r0, c0, r1, c1) in enumerate([
                (xr, 1, yr, 1), (xr, 1, yr, -1), (xi, 1, yi, 1),
                (xi, 1, yi, -1), (xr, 1, yi, 1), (xr, 1, yi, -1),
                (xi, 1, yr, 1), (xi, 1, yr, -1)]):
            sel[r0, 8 * q + j] += c0
            sel[r1, 8 * q + j] += c1
    C["SELPSQ"] = sel

    # SELP [96, 106]: pair-cmul operand pack (no-square-trick path):
    # rows 0:10 = x_q (r), 32:42 = y_q (r), 64:74 = x_q (i), 96:106 = y_q (i)
    selp = np.zeros((M_ROWS, M_ROWS_PP), np.float32)
    for q, (x, y) in enumerate(PAIR_XY):
        selp[x, q] = 1.0
        selp[y, M_PP + q] = 1.0
        selp[MI + x, MI + q] = 1.0
        selp[MI + y, MI + M_PP + q] = 1.0
    C["SELP"] = selp

    # L1 packs, 96 cols: r-parts 0:48 (m = 16g + c2), i-parts 48:96.
    # L1A = x operand; L1BN = [yr; yi]; L1BX = [yi; yr] (swapped) so the two
    # raw-product tiles M1 = b1*a, M2 = b2*a give t48r = M1[j]-M1[48+j],
    # t48i = M2[j]+M2[48+j].
    for side in "HG":
        packs = {}
        for nmpk, qoff in (("L1A", 0), ("L1B", 1)):
            colsr, colsi = [], []
            for g in range(3):
                for c2 in range(16):
                    z = (c2 % 4) if qoff == 0 else (c2 // 4)
                    fr, fi = tab_entry(side, 3 * g + qoff, z)
                    colsr.append(fr)
                    colsi.append(fi)
            packs[nmpk] = (colsr, colsi)
        ar, ai = packs["L1A"]
        br, bi = packs["L1B"]
        C[f"L1AM_{side}"], C[f"L1AS_{side}"] = _forms_to_mats(ar + ai)
        C[f"L1BNM_{side}"], C[f"L1BNS_{side}"] = _forms_to_mats(br + bi)
        C[f"L1BXM_{side}"], C[f"L1BXS_{side}"] = _forms_to_mats(bi + br)

    # REP01 [48, 128]: col 64g+c <- t48 row 16g + (c%16), g in {0,1}
    rep = np.zeros((48, 128), np.float32)
    for g in range(2):
        for c in range(64):
            rep[16 * g + (c % 16), 64 * g + c] = 1.0
    C["REP01"] = rep
    # REP2R/REP2I [48, 128]: col c / 64+c <- t48 row 32 + (c%16)
    r2r = np.zeros((48, 128), np.float32)
    r2i = np.zeros((48, 128), np.float32)
    for c in range(64):
        r2r[32 + (c % 16), c] = 1.0
        r2i[32 + (c % 16), 64 + c] = 1.0
    C["REP2R"], C["REP2I"] = r2r, r2i
    # N-variants read the raw-product tiles M1/M2 [96 rows] directly with the
    # quad +/- combine folded in: t48r = M1[0:48] - M1[48:96],
    # t48i = M2[0:48] + M2[48:96].
    C["REP01RN"] = np.concatenate([rep, -rep], 0)        # [96, 128] over M1
    C["REP01IN"] = np.concatenate([rep, rep], 0)         # [96, 128] over M2
    C["REP2RN"] = np.concatenate([r2r, -r2r], 0)         # [96, 128] over M1
    C["REP2IN"] = np.concatenate([r2i, r2i], 0)          # [96, 128] over M2

    # TABC packs
    for side in "HG":
        colsr, colsi = [], []
        for g in range(2):
            for c in range(64):
                fr, fi = tab_entry(side, 3 * g + 2, c // 16)
                colsr.append(fr)
                colsi.append(fi)
        C[f"TABC01RM_{side}"], C[f"TABC01RS_{side}"] = _forms_to_mats(colsr)
        C[f"TABC01IM_{side}"], C[f"TABC01IS_{side}"] = _forms_to_mats(colsi)
        cols = []
        for c in range(64):
            cols.append(tab2_entry(side, c // 16)[0])
        for c in range(64):
            cols.append(tab2_entry(side, c // 16)[1])
        C[f"TABC2M_{side}"], C[f"TABC2S_{side}"] = _forms_to_mats(cols)
    return C


CONSTS = _build_consts()


# ---------------------------------------------------------------- host inputs
LT_W = 192 + 2 * SQROWS                # 352: + square-input fold blocks


def build_ltpack(Ar, Ai):
    """[22, 352]: H block cols 0:96, G block cols 96:192 (rows 0:16),
    square-input folds (LT @ SELPSQ) at 192:272 (H) / 272:352 (G)."""
    lt = np.zeros((22, LT_W), np.float32)
    for side, kb, lo, base in (("H", 11, 0, 0), ("G", 7, 11, 96)):
        for nm, A in (("r", Ar), ("i", Ai)):
            ro = 0 if nm == "r" else MI
            At = A.T.astype(np.float32)
            # row-sum columns: lt[k, base+ro+i] = A[i, lo+k]
            lt[0:kb, base + ro:base + ro + N] = At[lo:lo + kb, :]
            # scaled rows SE/SO: sums of A[16,:]/A[17,:] against scaled masks
            lt[kb:2 * kb, base + ro + M_SE] = At[lo:lo + kb, 16]
            lt[kb:2 * kb, base + ro + M_SO] = At[lo:lo + kb, 17]
        if side == "H":
            lt[0, base + M_ONE] = 1.0     # MFX row 0 = ones
            lt[11, base + M_SGN] = 1.0    # MFX row 11 = sF
        else:
            lt[14, base + M_ONE] = 1.0    # MPX row 14 = ones
            lt[15, base + M_SGN] = 1.0    # MPX row 15 = sP
    sq = CONSTS["SELPSQ"]                 # [96, 80]
    lt[0:22, 192:272] = lt[0:22, 0:96] @ sq
    lt[0:16, 272:352] = lt[0:16, 96:192] @ sq
    return lt


# Master-folded LTPACK v2: for each table pack, the M-part selection matrix is
# folded into the per-input lt block on the host, so the device pack matmuls
# read (lt @ selM) directly against the exact +/-1 MASK tile. This removes the
# on-device master tile and its fp32r rounding (the dominant error source).
_FOLD_PACKS = [("L1A", 96), ("L1BN", 96), ("L1BX", 96),
               ("TABC01R", 128), ("TABC01I", 128), ("TABC2", 128)]
LT2_OFF = {}
_off = 160                              # 0:80 sqH fold, 80:160 sqG fold
for _nm, _m in _FOLD_PACKS:
    LT2_OFF[(_nm, "H")] = _off
    _off += _m
    LT2_OFF[(_nm, "G")] = _off
    _off += _m
LT2_W = _off                            # 1504


def build_ltpack2(Ar, Ai):
    lt = build_ltpack(Ar, Ai)
    out = np.zeros((22, LT2_W), np.float32)
    out[0:22, 0:80] = lt[0:22, 192:272]
    out[0:16, 80:160] = lt[0:16, 272:352]
    for nm, m in _FOLD_PACKS:
        oh, og = LT2_OFF[(nm, "H")], LT2_OFF[(nm, "G")]
        out[0:22, oh:oh + m] = lt[0:22, 0:96] @ CONSTS[f"{nm}M_H"][0:96]
        out[0:16, og:og + m] = lt[0:16, 96:192] @ CONSTS[f"{nm}M_G"][0:96]
    return out


# ---------------------------------------------------------------- emulator
def emulate(Ar, Ai, verbose=False):
    """Numpy mirror of the device computation (fp32/fp64 mix, no rounding)."""
    C = CONSTS
    ranges = {}
    lt = build_ltpack(Ar, Ai)
    masterw = np.zeros((M_ROWS_PP, W), np.float32)
    master = masterw[0:M_ROWS]
    master[:, 0:F] = lt[0:22, 0:96].T @ C["MASK"][0:22, 0:F]
    master[:, F:W] = lt[0:16, 96:192].T @ C["MASK"][0:16, F:W]
    ranges["master"] = np.abs(master).max()

    if USE_SQ:
        sq = (C["SELPSQ"].T @ master) ** 2                   # [80, W]
        ranges["sq"] = np.abs(sq).max()
    else:
        sq = np.zeros((SQROWS, W), np.float32)
        pk = C["SELP"].T @ master                            # [106, W]
        pr = pk[0:10] * pk[M_PP:M_PP + 10] - pk[MI:MI + 10] * pk[96:106]
        pi = pk[0:10] * pk[96:106] + pk[MI:MI + 10] * pk[M_PP:M_PP + 10]
        masterw[M_PP:M_PP + 10] = pr
        masterw[96:106] = pi
        ranges["pp"] = max(np.abs(pr).max(), np.abs(pi).max())

    def pack(nm):
        out = np.zeros((C[f"{nm}M_H"].shape[1], W), np.float32)
        for lo, hi, side in [(0, F, "H"), (F, W, "G")]:
            out[:, lo:hi] = (C[f"{nm}M_{side}"][0:M_ROWS_PP].T
                             @ masterw[:, lo:hi]
                             + C[f"{nm}S_{side}"].T @ sq[:, lo:hi])
        return out

    l1a, b1, b2 = pack("L1A"), pack("L1BN"), pack("L1BX")
    m1 = b1 * l1a                                            # [96, W]
    m2 = b2 * l1a
    q01r = C["REP01RN"].T @ m1                               # [128, W]
    q01i = C["REP01IN"].T @ m2
    q2 = C["REP2RN"].T @ m1 + C["REP2IN"].T @ m2             # [128, W]

    def packc(nms):
        m = C[f"{nms}M_H"].shape[1]
        out = np.zeros((m, W), np.float32)
        for lo, hi, side in [(0, F, "H"), (F, W, "G")]:
            out[:, lo:hi] = (C[f"{nms}M_{side}"].T @ masterw[:, lo:hi]
                             + C[f"{nms}S_{side}"].T @ sq[:, lo:hi])
        return out

    c01r, c01i = packc("TABC01R"), packc("TABC01I")
    c2 = packc("TABC2")

    er01 = q01r * c01r - q01i * c01i                         # [128, W]
    ei01 = q01r * c01i + q01i * c01r
    er2 = q2[0:64] * c2[0:64] - q2[64:128] * c2[64:128]      # [64, W]
    ei2 = q2[0:64] * c2[64:128] + q2[64:128] * c2[0:64]

    def tmm(er, ei, rows):
        gr = er[rows, F:W]                                   # [64, P]
        gi = ei[rows, F:W]
        hr = er[rows, 0:F]                                   # [64, F]
        hi = ei[rows, 0:F]
        tr = gr.T @ hr - gi.T @ hi                           # [P, F]
        ti = gr.T @ hi + gi.T @ hr
        return tr, ti

    t0 = tmm(er01, ei01, slice(0, 64))
    t1 = tmm(er01, ei01, slice(64, 128))
    t2 = tmm(er2, ei2, slice(0, 64))

    p01r = t0[0] * t1[0] - t0[1] * t1[1]
    p01i = t0[0] * t1[1] + t0[1] * t1[0]
    perm_r = float(np.sum(p01r * t2[0]) - np.sum(p01i * t2[1]))
    perm_i = float(np.sum(p01r * t2[1]) + np.sum(p01i * t2[0]))
    pa2 = np.float32(perm_r) ** 2 + np.float32(perm_i) ** 2
    return EMU * SCALE2 * pa2 + DARK


# ---------------------------------------------------------------- device
FP16 = mybir.dt.float16

# CPKH: fp32 selection constants, column-packed [128, *]. M-parts read the
# master tile (96 rows), S-parts the squares tile (80), REP packs m1/m2 (96).
_CPKH_ORDER = ["REP01RN", "REP01IN", "REP2RN", "REP2IN",
               "L1AS_H", "L1AS_G", "L1BNS_H", "L1BNS_G", "L1BXS_H", "L1BXS_G",
               "TABC01RS_H", "TABC01RS_G", "TABC01IS_H", "TABC01IS_G",
               "TABC2S_H", "TABC2S_G"]


def _build_cpkh():
    # t48 combined tile [112, W]: r rows 0:48, i rows 64:112. REP lhsTs pick
    # from it: REP01R/REP01I [112, 128] (quad replication for groups 0,1 from
    # r / i rows), REP2RI [112, 128] (g2: cols 0:64 <- r rows, 64:128 <- i).
    C = dict(CONSTS)
    rep01 = C["REP01"]                     # [48, 128]
    r01r = np.zeros((112, 128), np.float32)
    r01r[0:48, :] = rep01
    r01i = np.zeros((112, 128), np.float32)
    r01i[64:112, :] = rep01
    C["REP01R"], C["REP01I"] = r01r, r01i
    r2 = np.zeros((112, 128), np.float32)
    r2[0:48, :] = C["REP2R"]
    r2[64:112, 0:128] = C["REP2I"]
    C["REP2RI"] = r2

    cols = {}
    blocks = []
    off = 0
    for nm in _CPKH_ORDER:
        arr = C[nm]
        a = np.zeros((128, arr.shape[1]), np.float32)
        a[0:arr.shape[0], :] = arr
        cols[nm] = (off, arr.shape[1], arr.shape[0])
        blocks.append(a)
        off += arr.shape[1]
    return np.concatenate(blocks, axis=1).astype(np.float32), cols


CPKH, CPKH_COLS = _build_cpkh()


def host_inputs(A_real_b: np.ndarray, A_imag_b: np.ndarray) -> dict:
    """Per-core input map for one batch element (pure reindexing of A)."""
    return {"LTPACK": build_ltpack2(A_real_b, A_imag_b),
            "MASKC": CONSTS["MASK"], "CPKH": CPKH}


def build_kernel(loop_iters=None):
    nc = bacc.Bacc("TRN2", target_bir_lowering=False, debug=False)
    tens = {}
    tens["LTPACK"] = nc.dram_tensor("LTPACK", [22, LT2_W], FP32R,
                                    kind="ExternalInput").ap()
    tens["MASKC"] = nc.dram_tensor("MASKC", [22, W], FP32R,
                                   kind="ExternalInput").ap()
    tens["CPKH"] = nc.dram_tensor("CPKH", [128, CPKH.shape[1]], FP32R,
                                  kind="ExternalInput").ap()
    tens["OUT"] = nc.dram_tensor("OUT", [1, 1], FP32, kind="ExternalOutput").ap()

    with tile.TileContext(nc) as tc:
        from contextlib import ExitStack
        ctx = ExitStack()
        pools = (ctx.enter_context(tc.tile_pool(name="pers", bufs=1)),
                 ctx.enter_context(tc.tile_pool(name="cm", bufs=2)),
                 ctx.enter_context(tc.tile_pool(name="psum", bufs=2,
                                                space="PSUM")))
        if loop_iters is None:
            tiles = _load(nc, pools, tens)
            _body(nc, tc, tens, pools, tiles)
        else:
            # timing build: reload + full body each iteration so the
            # loop-slope measurement reflects a complete dispatch
            with tc.For_i(0, loop_iters, 1):
                tiles = _load(nc, pools, tens)
                _body(nc, tc, tens, pools, tiles)
        ctx.close()
    nc.compile()
    return nc


def _load(nc, pools, tens):
    """Input DMAs: loop-invariant, issued once outside any timing loop."""
    pers, _cm, _psum = pools
    dma = nc.sync.dma_start
    cpk32 = pers.tile([22, W], FP32R, tag="cpk32")
    for lo, hi, _s in CHUNKS:
        dma(cpk32[:, lo:hi], tens["MASKC"][:, lo:hi])
    ltp = pers.tile([22, LT2_W], FP32R, tag="ltp")
    lt_l1 = LT2_OFF[("TABC01R", "H")]
    nc.scalar.dma_start(ltp[:, 0:160], tens["LTPACK"][:, 0:160])
    dma(ltp[:, 160:lt_l1], tens["LTPACK"][:, 160:lt_l1])
    dma(ltp[:, lt_l1:LT2_W], tens["LTPACK"][:, lt_l1:LT2_W])
    cpkh = pers.tile([128, CPKH.shape[1]], FP32R, tag="cpkh")
    nc.gpsimd.dma_start(cpkh[:], tens["CPKH"][:, :])
    return cpk32, ltp, cpkh


def _body(nc, tc, tens, pools, tiles):
    pers, cm, psum_pool = pools
    cpk32, ltp, cpkh = tiles
    dma = nc.sync.dma_start

    def csel(name, krows):
        off, width, _ = CPKH_COLS[name]
        return cpkh[0:krows, off:off + width]

    def emul(eng, out, a, b):
        eng.tensor_mul(out, a, b)

    def eaddsub(eng, out, a, b, op):
        if op == OP.add:
            eng.tensor_add(out, a, b)
        else:
            eng.tensor_sub(out, a, b)

    def cmul6(rows, w, i0, i1, outr, outi, dt=FP32, engs=None):
        """Complex multiply; engs = 6-tuple of engines for op distribution."""
        if engs is None:
            engs = (nc.vector,) * 6
        i0r, i0i = i0
        i1r, i1i = i1
        t1 = cm.tile([rows, w], dt, tag="cm_t1")
        t2 = cm.tile([rows, w], dt, tag="cm_t2")
        t3 = cm.tile([rows, w], dt, tag="cm_t3")
        t4 = cm.tile([rows, w], dt, tag="cm_t4")
        emul(engs[0], t1[:], i0r, i1r)
        emul(engs[1], t2[:], i0i, i1i)
        eaddsub(engs[2], outr, t1[:], t2[:], OP.subtract)
        emul(engs[3], t3[:], i0r, i1i)
        emul(engs[4], t4[:], i0i, i1r)
        eaddsub(engs[5], outi, t3[:], t4[:], OP.add)

    def R(ap):
        return ap.bitcast(FP32R)

    def sel_mm(const_name, rhs_tile, m, krows, rhs_rows, per_side=True,
               s_name=None, sq_tile=None):
        ps = psum_pool.tile([m, W], FP32, tag="ps")
        for lo, hi, side in CHUNKS:
            key = f"{const_name}_{side}" if per_side else const_name
            nc.tensor.matmul(ps[:, lo:hi], R(csel(key, krows)),
                             R(rhs_tile[0:rhs_rows, lo:hi]),
                             start=True, stop=(s_name is None))
            if s_name is not None:
                skey = f"{s_name}_{side}"
                nc.tensor.matmul(ps[:, lo:hi], R(csel(skey, SQROWS)),
                                 R(sq_tile[0:SQROWS, lo:hi]),
                                 start=False, stop=True)
        return ps

    def evac(ps, rows, w, tag, dt=FP32, eng=None):
        sb = pers.tile([rows, w], dt, tag=tag)
        if eng is nc.vector:
            eng.tensor_copy(sb[:], ps[0:rows, :])
        else:
            (eng or nc.scalar).copy(sb[:], ps[0:rows, :])
        return sb

    def evac3(ps, rows, tag, dt=FP32):
        """Per-chunk evac: each chunk lands as soon as its matmul is done."""
        sb = pers.tile([rows, W], dt, tag=tag)
        for lo, hi, _s in CHUNKS:
            nc.scalar.copy(sb[:, lo:hi], ps[0:rows, lo:hi])
        return sb


    # ---- square-input sums (folds in LTPACK cols 0:160), ACT Square evac;
    # the master row-sums are folded into the pack lhsT columns on the host
    ps_sq = psum_pool.tile([SQROWS, W], FP32, tag="ps")
    for lo, hi, side in CHUNKS:
        if side == "H":
            nc.tensor.matmul(ps_sq[:, lo:hi], ltp[0:22, 0:80],
                             cpk32[0:22, lo:hi], start=True, stop=True)
        else:
            nc.tensor.matmul(ps_sq[:, lo:hi], ltp[0:16, 80:160],
                             cpk32[0:16, lo:hi], start=True, stop=True)
    sqt = pers.tile([SQROWS, W], FP32R, tag="sqt")
    nc.scalar.activation(sqt[:], ps_sq[0:SQROWS, :], AF.Square)

    def fpack(name, m, s_name):
        ps = psum_pool.tile([m, W], FP32, tag="ps")
        oh, og = LT2_OFF[(name, "H")], LT2_OFF[(name, "G")]
        for lo, hi, side in CHUNKS:
            if side == "H":
                nc.tensor.matmul(ps[:, lo:hi], ltp[0:22, oh:oh + m],
                                 cpk32[0:22, lo:hi], start=True, stop=False)
            else:
                nc.tensor.matmul(ps[:, lo:hi], ltp[0:16, og:og + m],
                                 cpk32[0:16, lo:hi], start=True, stop=False)
            nc.tensor.matmul(ps[:, lo:hi], R(csel(f"{s_name}_{side}", SQROWS)),
                             R(sqt[0:SQROWS, lo:hi]), start=False, stop=True)
        return ps

    # ---- quad layer: raw-product tiles m1/m2 [96, W]; the +/- combine into
    # t48 quantities is folded into the REP*N constants downstream.
    ps_a = fpack("L1A", 96, "L1AS")
    a_sb = evac(ps_a, 96, W, "a_sb")
    ps_b1 = fpack("L1BN", 96, "L1BNS")
    ps_b2 = fpack("L1BX", 96, "L1BXS")
    # m1/m2 stay exact fp32; the q packs below run as plain fp32 matmuls
    m1 = pers.tile([96, W], FP32, tag="m1")
    m2 = pers.tile([96, W], FP32, tag="m2")
    nc.vector.tensor_mul(m1[:], ps_b1[0:96, :], a_sb[:])
    nc.vector.tensor_mul(m2[:], ps_b2[0:96, :], a_sb[:])

    # ---- L2 operand packs; TABC evac'd, q01 consumed from PSUM
    c01r = evac(fpack("TABC01R", 128, "TABC01RS"), 128, W, "c01r")
    c01i = evac(fpack("TABC01I", 128, "TABC01IS"), 128, W, "c01i")
    # ---- L2 raw products; q01 packs first (01 path is the critical chain)
    def qpack(cname, rhs):
        ps = psum_pool.tile([128, W], FP32, tag="ps")
        for lo, hi, _s in CHUNKS:
            nc.tensor.matmul(ps[:, lo:hi], csel(cname, 96).bitcast(FP32),
                             rhs[0:96, lo:hi], start=True, stop=True)
        return ps

    ps_q01r = qpack("REP01RN", m1)
    ps_q01i = qpack("REP01IN", m2)
    q01i = evac(ps_q01i, 128, W, "q01i")
    c2 = evac(fpack("TABC2", 128, "TABC2S"), 128, W, "c2", eng=nc.vector)
    t1a = cm.tile([128, W], FP32, tag="cm_t1")
    t2a = cm.tile([128, W], FP32, tag="cm_t2")
    t3a = cm.tile([128, W], FP32, tag="cm_t3")
    t4a = cm.tile([128, W], FP32, tag="cm_t4")
    emul(nc.vector, t1a[:], ps_q01r[:], c01r[:])
    emul(nc.gpsimd, t2a[:], q01i[:], c01i[:])
    emul(nc.vector, t3a[:], ps_q01r[:], c01i[:])
    emul(nc.gpsimd, t4a[:], q01i[:], c01r[:])
    er01 = pers.tile([128, W], FP32R, tag="er01")
    ei01 = pers.tile([128, W], FP32R, tag="ei01")
    eaddsub(nc.vector, er01[:], t1a[:], t2a[:], OP.subtract)
    eaddsub(nc.vector, ei01[:], t3a[:], t4a[:], OP.add)

    ps_q2 = psum_pool.tile([128, W], FP32, tag="ps")
    for lo, hi, _s in CHUNKS:
        nc.tensor.matmul(ps_q2[:, lo:hi], csel("REP2RN", 96).bitcast(FP32),
                         m1[0:96, lo:hi], start=True, stop=False)
        nc.tensor.matmul(ps_q2[:, lo:hi], csel("REP2IN", 96).bitcast(FP32),
                         m2[0:96, lo:hi], start=False, stop=True)
    q2 = evac(ps_q2, 128, W, "q2")
    t1b = cm.tile([64, W], FP32, tag="cm_t1")
    t2b = cm.tile([64, W], FP32, tag="cm_t2")
    t3b = cm.tile([64, W], FP32, tag="cm_t3")
    t4b = cm.tile([64, W], FP32, tag="cm_t4")
    emul(nc.gpsimd, t1b[:], q2[0:64, :], c2[0:64, :])
    emul(nc.gpsimd, t2b[:], q2[64:128, :], c2[64:128, :])
    emul(nc.vector, t3b[:], ps_q2[0:64, :], c2[64:128, :])
    emul(nc.vector, t4b[:], ps_q2[64:128, :], c2[0:64, :])
    er2 = pers.tile([64, W], FP32R, tag="er2")
    ei2 = pers.tile([64, W], FP32R, tag="ei2")
    eaddsub(nc.vector, er2[:], t1b[:], t2b[:], OP.subtract)
    eaddsub(nc.gpsimd, ei2[:], t3b[:], t4b[:], OP.add)

    # stage 4 on views of er/ei; only the negated G-side imag needs a tile
    zer = pers.tile([128, P], FP32, tag="zer")
    nc.gpsimd.memset(zer[:], 0.0)
    eineg = pers.tile([128, P], FP32R, tag="eineg")
    nc.gpsimd.tensor_sub(eineg[:], zer[:], ei01[:, F:W])
    eineg2 = pers.tile([64, P], FP32R, tag="eineg2")
    nc.gpsimd.tensor_sub(eineg2[:], zer[0:64, :], ei2[:, F:W])

    def tmm(g, which, tg="ps", bf=None):
        if g < 2:
            sl = slice(64 * g, 64 * g + 64)
            src_r, src_i, neg = er01, ei01, eineg
        else:
            sl = slice(0, 64)
            src_r, src_i, neg = er2, ei2, eineg2
        gr, gi, gin = src_r[sl, F:W], src_i[sl, F:W], neg[sl, :]
        ps = psum_pool.tile([P, F], FP32, tag=tg, bufs=bf)
        for c0 in range(0, F, 512):
            c1 = c0 + 512
            if which == "r":   # Tr = gr^T hr - gi^T hi
                nc.tensor.matmul(ps[:, c0:c1], R(gr), R(src_r[sl, c0:c1]),
                                 start=True, stop=False)
                nc.tensor.matmul(ps[:, c0:c1], R(gin), R(src_i[sl, c0:c1]),
                                 start=False, stop=True)
            else:              # Ti = gi^T hr + gr^T hi
                nc.tensor.matmul(ps[:, c0:c1], R(gi), R(src_r[sl, c0:c1]),
                                 start=True, stop=False)
                nc.tensor.matmul(ps[:, c0:c1], R(gr), R(src_i[sl, c0:c1]),
                                 start=False, stop=True)
        return ps

    # ---- T0 evac'd; T1 consumed from PSUM by the p01 multiplies; T2 last
    b0r = evac(tmm(0, "r", "t4", 1), P, F, "B0r")
    b0i = evac(tmm(0, "i"), P, F, "B0i")
    b1r = evac(tmm(1, "r"), P, F, "B1r")
    ps_t1i = tmm(1, "i", "t4", 1)

    t1p = cm.tile([P, F], FP32, tag="cm_t1")
    t2p = cm.tile([P, F], FP32, tag="cm_t2")
    t3p = cm.tile([P, F], FP32, tag="cm_t3")
    t4p = cm.tile([P, F], FP32, tag="cm_t4")
    nc.vector.tensor_mul(t2p[:], ps_t1i[:], b0i[:])
    nc.vector.tensor_mul(t4p[:], ps_t1i[:], b0r[:])
    emul(nc.gpsimd, t1p[:], b1r[:], b0r[:])
    emul(nc.gpsimd, t3p[:], b1r[:], b0i[:])
    p01r = pers.tile([P, F], FP32, tag="p01r")
    p01i = pers.tile([P, F], FP32, tag="p01i")
    nc.vector.tensor_sub(p01r[:], t1p[:], t2p[:])
    eaddsub(nc.gpsimd, p01i[:], t3p[:], t4p[:], OP.add)

    ps_t2r = tmm(2, "r")
    ps_t2i = tmm(2, "i")

    scr1 = pers.tile([P, F], FP32, tag="scr1")
    scr2 = pers.tile([P, F], FP32, tag="scr2")
    scr3 = pers.tile([P, F], FP32, tag="scr3")
    scr4 = pers.tile([P, F], FP32, tag="scr4")
    acc = pers.tile([P, 4], FP32, tag="acc")
    nc.vector.scalar_tensor_tensor(
        out=scr1[:], in0=p01r[:], scalar=1.0, in1=ps_t2r[:],
        op0=OP.mult, op1=OP.mult, accum_out=acc[:, 0:1])
    nc.vector.scalar_tensor_tensor(
        out=scr3[:], in0=p01r[:], scalar=1.0, in1=ps_t2i[:],
        op0=OP.mult, op1=OP.mult, accum_out=acc[:, 2:3])
    nc.vector.scalar_tensor_tensor(
        out=scr2[:], in0=p01i[:], scalar=-1.0, in1=ps_t2i[:],
        op0=OP.mult, op1=OP.mult, accum_out=acc[:, 1:2])
    nc.vector.scalar_tensor_tensor(
        out=scr4[:], in0=p01i[:], scalar=1.0, in1=ps_t2r[:],
        op0=OP.mult, op1=OP.mult, accum_out=acc[:, 3:4])

    ones = pers.tile([P, 1], FP32, tag="ones")
    nc.gpsimd.memset(ones[:], 1.0)
    dark = pers.tile([1, 1], FP32, tag="dark")
    nc.gpsimd.memset(dark[:], float(DARK))
    accs = pers.tile([P, 2], FP32, tag="accs")
    nc.vector.tensor_add(accs[:, 0:1], acc[:, 0:1], acc[:, 1:2])
    nc.vector.tensor_add(accs[:, 1:2], acc[:, 2:3], acc[:, 3:4])
    red = psum_pool.tile([1, 2], FP32, tag="t4", bufs=1)
    nc.tensor.matmul(red[:], ones[:], accs[:], start=True, stop=True)

    p2 = pers.tile([1, 2], FP32, tag="p2")
    nc.scalar.activation(p2[:], red[:], AF.Square)
    pa2 = pers.tile([1, 1], FP32, tag="pa2")
    nc.vector.reduce_sum(pa2[:], p2[:], axis=mybir.AxisListType.X)
    res = pers.tile([1, 1], FP32, tag="res")
    nc.vector.scalar_tensor_tensor(
        out=res[:], in0=pa2[:], scalar=float(EMU * SCALE2), in1=dark[:],
        op0=OP.mult, op1=OP.add)
    dma(tens["OUT"][:, :], res[:])


def kernel(A_real: np.ndarray, A_imag: np.ndarray) -> np.ndarray:
    Bn = A_real.shape[0]
    assert Bn == 8 and A_real.shape == (Bn, N, N)
    if "nc" not in _CACHE:
        _CACHE["nc"] = build_kernel()
    nc = _CACHE["nc"]
    in_maps = [host_inputs(np.asarray(A_real[b], dtype=np.float32),
                           np.asarray(A_imag[b], dtype=np.float32))
               for b in range(Bn)]
    res = run_bass_kernel_spmd(nc, in_maps, list(range(Bn)))
    out = np.array([res.results[b]["OUT"].reshape(-1)[0] for b in range(Bn)],
                   dtype=np.float32)
    return out


def ref_prob(ar, ai):
    n = N
    idx = np.arange(2 ** n, dtype=np.uint32)
    mask = ((idx[:, None] >> np.arange(n, dtype=np.uint32)[None, :]) & 1
            ).astype(np.float32)
    k = mask.sum(axis=1)
    sign = np.where((n - k) % 2 == 0, 1.0, -1.0).astype(np.complex64)
    rs = (mask @ ar.T.astype(np.float32)
          + 1j * (mask @ ai.T.astype(np.float32))).astype(np.complex64)
    prod = rs[:, 0].copy()
    for i in range(1, n):
        prod = prod * rs[:, i]
    perm = np.sum(prod * sign)
    pa2 = np.float32(perm.real) ** 2 + np.float32(perm.imag) ** 2
    return EMU * pa2 + DARK


if __name__ == "__main__":
    rng = np.random.default_rng(0)
    for t in range(3):
        Ar = (rng.standard_normal((N, N)) / np.sqrt(N)).astype(np.float32)
        Ai = (rng.standard_normal((N, N)) / np.sqrt(N)).astype(np.float32)
        got = emulate(Ar, Ai)
        want = ref_prob(Ar, Ai)
        print(f"emu={got:.6e} ref={want:.6e} rel={abs(got-want)/abs(want):.2e}")



# revision 83
# speedup vs baseline: 3.3553x; 3.3553x over previous
"""Trainium2 Bass kernel: boson-sampler probabilities via Glynn's permanent.

Glynn with 2^17 terms on a [P=128, F=1024] grid; rows grouped 6+6+6 into
three rank-64 tables contracted against the grid (stage 4), then the
T0*T1*T2 signed reduce. Unified-width tiles [rows, 1152] carry the H (1024)
and G (128) sides through shared ops with per-512-chunk matmul constants.

Speed structure (vs the fp32 baseline):
- All selection/contraction matmuls run fp32r (1 cyc/col vs 4 for fp32).
  fp32r is ~10-bit-mantissa; anything consumed by an fp32r matmul must be
  *written* as fp32r (verifier rule), which rounds it. The final Glynn
  cancellation amplifies pre-reduction rounding ~100x, so:
  * the master row-sum stage is folded into per-input host lhsT columns
    (LTPACK blocks) multiplied against the exact +/-1 MASK tile - no
    on-device master tile, no rounding, and the whole RS stage disappears;
  * pair products use the ACT-Square trick (squares of host-folded sums),
    one rounded stage (sqt);
  * the quad layer is two raw product tiles m1/m2 kept in exact fp32, with
    the +/- combine folded into the REP* constants of plain-fp32 q-pack
    matmuls (4 cyc/col, paid for accuracy);
  * only the L2 tables (er/ei) remain rounded -> measured ~9e-3 rel err on
    HW vs the 2e-2 gate.
- Engine legality learned from walrus: GPSIMD touches SBUF only (plain
  tensor ops, no STT); at most one PSUM operand per DVE op; both-SBUF
  operands need equal base partitions; engine writes start at partition
  0/32/64/96. Products pair one PSUM with one SBUF operand where possible;
  Pool gets equal-base SBUF products fed by ACT evacuations.
- Stage 4 reads table tiles through views (G cols as lhsT, H cols as rhs)
  with two accumulating K=64 matmuls per chunk - no repack stage.
- Final: p01 = T0*T1 (DVE/Pool), then a Karatsuba reduce in THREE fused
  scalar_tensor_tensor accumulations: m1 = sum p01r*T2r, m2 = sum p01i*T2i,
  m3 = sum (p01r+p01i)*(T2r+T2i), with T2r+T2i accumulated for free in PSUM
  by an extra 8-matmul group; perm = (m1-m2, m3-m1-m2). Ones-matmul reduce,
  ACT-Square, one STT tail.
One NeuronCore per batch element; input DMAs split across the SP and ACT
queues (LTPACK in three pieces) so the ACT Square is never queue-blocked.
"""

import sys

sys.path.insert(0, "/opt/trn_rl_repo")

import numpy as np

import concourse.bacc as bacc
import concourse.bass as bass
import concourse.tile as tile
from concourse import mybir
from concourse.bass_utils import run_bass_kernel_spmd

FP32 = mybir.dt.float32
FP32R = mybir.dt.float32r
BF16 = mybir.dt.bfloat16
OP = mybir.AluOpType
AF = mybir.ActivationFunctionType

N = 18
PBITS, FBITS = 7, 10
P, F = 1 << PBITS, 1 << FBITS          # 128, 1024
W = F + P                              # 1152 unified width
EMU = 0.85 * (1 - 0.02) * (1 - 0.02) * (1 - 0.01)
DARK = 1e-6 * N
SCALE2 = float(2.0 ** (2 * (1 - N)))

# master rows (one tile): r-block base 0, i-block base 64
M_SE, M_SO, M_ONE, M_SGN = 18, 19, 20, 21
MI = 64                                # imag row offset
M_PP = 32                              # pair-product rows 32:42 (r), 96:106 (i)
M_ROWS = 96                            # rows written by the RS matmul
M_ROWS_PP = 106                        # master height incl. pair products
NQ = 10                                # pairs: 9 regular + 1 sign-scaled
SQROWS = 8 * NQ                        # 80
USE_SQ = True                          # pair products via ACT-square trick

CHUNKS = [(0, 512, "H"), (512, 1024, "H"), (1024, 1152, "G")]

_CACHE = {}


def _pm_mask(nvals, bits):
    v = np.arange(nvals, dtype=np.uint32)
    m = (v[:, None] >> np.arange(bits, dtype=np.uint32)[None, :]) & 1
    return (1.0 - 2.0 * m).astype(np.float32).T.copy()   # [bits, nvals]


def _parity(nvals, bits):
    v = np.arange(nvals, dtype=np.uint32)
    pc = np.zeros(nvals, dtype=np.uint32)
    for k in range(bits):
        pc += (v >> k) & 1
    return np.where(pc % 2 == 0, 1.0, -1.0).astype(np.float32)


# ---------------------------------------------------------------- linear forms
# A table entry is a pair (r_form, i_form); each form is a linear combination
# {"m": {master_row: coef}, "s": {sqp_row: coef}} over master / squares tiles.
PAIR_XY = [(2 * q, 2 * q + 1) for q in range(9)] + [(M_SE, 17)]


def _pp_form(q):
    b = 8 * q
    r = {"s": {b + 0: .25, b + 1: -.25, b + 2: -.25, b + 3: .25}}
    i = {"s": {b + 4: .25, b + 5: -.25, b + 6: .25, b + 7: -.25}}
    return (r, i)


def _row_form(k):
    return ({"m": {k: 1.0}}, {"m": {MI + k: 1.0}})


_ONES_F = ({"m": {M_ONE: 1.0}}, {})
_SGN_F = ({"m": {M_SGN: 1.0}}, {})


def tab_entry(side, q, z):
    """Pair-table entry (pair q, z in 0..3) as linear forms."""
    if side == "H":
        m = {0: "PP", 1: "odd", 2: "even", 3: "one"}[z]
    else:
        m = {0: "one", 1: "even", 2: "odd", 3: "PP"}[z]
    if m == "one":
        return _ONES_F
    if m == "PP":
        return _pp_form(q) if USE_SQ else _row_form(M_PP + q)
    k = 2 * q + (1 if m == "odd" else 0)
    return _row_form(k)


def tab2_entry(side, z):
    """Sign-scaled pair-8 table entry (uses scaled rows / scaled pair 9)."""
    if side == "H":
        m = {0: "PPS", 1: "SO", 2: "SE", 3: "sgn"}[z]
    else:
        m = {0: "sgn", 1: "SE", 2: "SO", 3: "PPS"}[z]
    if m == "sgn":
        return _SGN_F
    if m == "PPS":
        return _pp_form(9) if USE_SQ else _row_form(M_PP + 9)
    return _row_form(M_SE if m == "SE" else M_SO)


def _forms_to_mats(cols, nm_rows=M_ROWS_PP, ns_rows=SQROWS):
    """cols: list of forms -> (master-part [96, m], sqp-part [80, m])."""
    m = len(cols)
    selm = np.zeros((nm_rows, m), np.float32)
    sels = np.zeros((ns_rows, m), np.float32)
    for j, form in enumerate(cols):
        for k, c in form.get("m", {}).items():
            selm[k, j] += c
        for k, c in form.get("s", {}).items():
            sels[k, j] += c
    return selm, sels


# ---------------------------------------------------------------- constants
def _build_consts():
    C = {}
    # masks
    mF = np.concatenate([np.ones((1, F), np.float32), _pm_mask(F, FBITS)], 0)
    sF = _parity(F, FBITS)
    MFX = np.concatenate([mF, mF * sF[None, :]], 0)              # [22, F]
    mP = _pm_mask(P, PBITS)
    sP = _parity(P, PBITS)
    MPX = np.concatenate(
        [mP, mP * sP[None, :], np.ones((1, P), np.float32), sP[None, :]], 0)
    MASK = np.zeros((22, W), np.float32)
    MASK[0:22, 0:F] = MFX
    MASK[0:16, F:W] = MPX
    C["MASK"] = MASK

    # SELPSQ [96, 80]: square-input sum rows, 8 per pair
    # (s1..s8) = (xr+yr, xr-yr, xi+yi, xi-yi, xr+yi, xr-yi, xi+yr, xi-yr)
    sel = np.zeros((M_ROWS, SQROWS), np.float32)
    for q, (x, y) in enumerate(PAIR_XY):
        xr, xi, yr, yi = x, MI + x, y, MI + y
        for j, (r0, c0, r1, c1) in enumerate([
                (xr, 1, yr, 1), (xr, 1, yr, -1), (xi, 1, yi, 1),
                (xi, 1, yi, -1), (xr, 1, yi, 1), (xr, 1, yi, -1),
                (xi, 1, yr, 1), (xi, 1, yr, -1)]):
            sel[r0, 8 * q + j] += c0
            sel[r1, 8 * q + j] += c1
    C["SELPSQ"] = sel

    # SELP [96, 106]: pair-cmul operand pack (no-square-trick path):
    # rows 0:10 = x_q (r), 32:42 = y_q (r), 64:74 = x_q (i), 96:106 = y_q (i)
    selp = np.zeros((M_ROWS, M_ROWS_PP), np.float32)
    for q, (x, y) in enumerate(PAIR_XY):
        selp[x, q] = 1.0
        selp[y, M_PP + q] = 1.0
        selp[MI + x, MI + q] = 1.0
        selp[MI + y, MI + M_PP + q] = 1.0
    C["SELP"] = selp

    # L1 packs, 96 cols: r-parts 0:48 (m = 16g + c2), i-parts 48:96.
    # L1A = x operand; L1BN = [yr; yi]; L1BX = [yi; yr] (swapped) so the two
    # raw-product tiles M1 = b1*a, M2 = b2*a give t48r = M1[j]-M1[48+j],
    # t48i = M2[j]+M2[48+j].
    for side in "HG":
        packs = {}
        for nmpk, qoff in (("L1A", 0), ("L1B", 1)):
            colsr, colsi = [], []
            for g in range(3):
                for c2 in range(16):
                    z = (c2 % 4) if qoff == 0 else (c2 // 4)
                    fr, fi = tab_entry(side, 3 * g + qoff, z)
                    colsr.append(fr)
                    colsi.append(fi)
            packs[nmpk] = (colsr, colsi)
        ar, ai = packs["L1A"]
        br, bi = packs["L1B"]
        C[f"L1AM_{side}"], C[f"L1AS_{side}"] = _forms_to_mats(ar + ai)
        C[f"L1BNM_{side}"], C[f"L1BNS_{side}"] = _forms_to_mats(br + bi)
        C[f"L1BXM_{side}"], C[f"L1BXS_{side}"] = _forms_to_mats(bi + br)

    # REP01 [48, 128]: col 64g+c <- t48 row 16g + (c%16), g in {0,1}
    rep = np.zeros((48, 128), np.float32)
    for g in range(2):
        for c in range(64):
            rep[16 * g + (c % 16), 64 * g + c] = 1.0
    C["REP01"] = rep
    # REP2R/REP2I [48, 128]: col c / 64+c <- t48 row 32 + (c%16)
    r2r = np.zeros((48, 128), np.float32)
    r2i = np.zeros((48, 128), np.float32)
    for c in range(64):
        r2r[32 + (c % 16), c] = 1.0
        r2i[32 + (c % 16), 64 + c] = 1.0
    C["REP2R"], C["REP2I"] = r2r, r2i
    # N-variants read the raw-product tiles M1/M2 [96 rows] directly with the
    # quad +/- combine folded in: t48r = M1[0:48] - M1[48:96],
    # t48i = M2[0:48] + M2[48:96].
    C["REP01RN"] = np.concatenate([rep, -rep], 0)        # [96, 128] over M1
    C["REP01IN"] = np.concatenate([rep, rep], 0)         # [96, 128] over M2
    C["REP2RN"] = np.concatenate([r2r, -r2r], 0)         # [96, 128] over M1
    C["REP2IN"] = np.concatenate([r2i, r2i], 0)          # [96, 128] over M2

    # TABC packs
    for side in "HG":
        colsr, colsi = [], []
        for g in range(2):
            for c in range(64):
                fr, fi = tab_entry(side, 3 * g + 2, c // 16)
                colsr.append(fr)
                colsi.append(fi)
        C[f"TABC01RM_{side}"], C[f"TABC01RS_{side}"] = _forms_to_mats(colsr)
        C[f"TABC01IM_{side}"], C[f"TABC01IS_{side}"] = _forms_to_mats(colsi)
        cols = []
        for c in range(64):
            cols.append(tab2_entry(side, c // 16)[0])
        for c in range(64):
            cols.append(tab2_entry(side, c // 16)[1])
        C[f"TABC2M_{side}"], C[f"TABC2S_{side}"] = _forms_to_mats(cols)
    return C


CONSTS = _build_consts()


# ---------------------------------------------------------------- host inputs
LT_W = 192 + 2 * SQROWS                # 352: + square-input fold blocks


def build_ltpack(Ar, Ai):
    """[22, 352]: H block cols 0:96, G block cols 96:192 (rows 0:16),
    square-input folds (LT @ SELPSQ) at 192:272 (H) / 272:352 (G)."""
    lt = np.zeros((22, LT_W), np.float32)
    for side, kb, lo, base in (("H", 11, 0, 0), ("G", 7, 11, 96)):
        for nm, A in (("r", Ar), ("i", Ai)):
            ro = 0 if nm == "r" else MI
            At = A.T.astype(np.float32)
            # row-sum columns: lt[k, base+ro+i] = A[i, lo+k]
            lt[0:kb, base + ro:base + ro + N] = At[lo:lo + kb, :]
            # scaled rows SE/SO: sums of A[16,:]/A[17,:] against scaled masks
            lt[kb:2 * kb, base + ro + M_SE] = At[lo:lo + kb, 16]
            lt[kb:2 * kb, base + ro + M_SO] = At[lo:lo + kb, 17]
        if side == "H":
            lt[0, base + M_ONE] = 1.0     # MFX row 0 = ones
            lt[11, base + M_SGN] = 1.0    # MFX row 11 = sF
        else:
            lt[14, base + M_ONE] = 1.0    # MPX row 14 = ones
            lt[15, base + M_SGN] = 1.0    # MPX row 15 = sP
    sq = CONSTS["SELPSQ"]                 # [96, 80]
    lt[0:22, 192:272] = lt[0:22, 0:96] @ sq
    lt[0:16, 272:352] = lt[0:16, 96:192] @ sq
    return lt


# Master-folded LTPACK v2: for each table pack, the M-part selection matrix is
# folded into the per-input lt block on the host, so the device pack matmuls
# read (lt @ selM) directly against the exact +/-1 MASK tile. This removes the
# on-device master tile and its fp32r rounding (the dominant error source).
_FOLD_PACKS = [("L1A", 96), ("L1BN", 96), ("L1BX", 96),
               ("TABC01R", 128), ("TABC01I", 128), ("TABC2", 128)]
LT2_OFF = {}
_off = 160                              # 0:80 sqH fold, 80:160 sqG fold
for _nm, _m in _FOLD_PACKS:
    LT2_OFF[(_nm, "H")] = _off
    _off += _m
    LT2_OFF[(_nm, "G")] = _off
    _off += _m
LT2_W = _off                            # 1504


def build_ltpack2(Ar, Ai):
    lt = build_ltpack(Ar, Ai)
    out = np.zeros((22, LT2_W), np.float32)
    out[0:22, 0:80] = lt[0:22, 192:272]
    out[0:16, 80:160] = lt[0:16, 272:352]
    for nm, m in _FOLD_PACKS:
        oh, og = LT2_OFF[(nm, "H")], LT2_OFF[(nm, "G")]
        out[0:22, oh:oh + m] = lt[0:22, 0:96] @ CONSTS[f"{nm}M_H"][0:96]
        out[0:16, og:og + m] = lt[0:16, 96:192] @ CONSTS[f"{nm}M_G"][0:96]
    return out


# ---------------------------------------------------------------- emulator
def emulate(Ar, Ai, verbose=False):
    """Numpy mirror of the device computation (fp32/fp64 mix, no rounding)."""
    C = CONSTS
    ranges = {}
    lt = build_ltpack(Ar, Ai)
    masterw = np.zeros((M_ROWS_PP, W), np.float32)
    master = masterw[0:M_ROWS]
    master[:, 0:F] = lt[0:22, 0:96].T @ C["MASK"][0:22, 0:F]
    master[:, F:W] = lt[0:16, 96:192].T @ C["MASK"][0:16, F:W]
    ranges["master"] = np.abs(master).max()

    if USE_SQ:
        sq = (C["SELPSQ"].T @ master) ** 2                   # [80, W]
        ranges["sq"] = np.abs(sq).max()
    else:
        sq = np.zeros((SQROWS, W), np.float32)
        pk = C["SELP"].T @ master                            # [106, W]
        pr = pk[0:10] * pk[M_PP:M_PP + 10] - pk[MI:MI + 10] * pk[96:106]
        pi = pk[0:10] * pk[96:106] + pk[MI:MI + 10] * pk[M_PP:M_PP + 10]
        masterw[M_PP:M_PP + 10] = pr
        masterw[96:106] = pi
        ranges["pp"] = max(np.abs(pr).max(), np.abs(pi).max())

    def pack(nm):
        out = np.zeros((C[f"{nm}M_H"].shape[1], W), np.float32)
        for lo, hi, side in [(0, F, "H"), (F, W, "G")]:
            out[:, lo:hi] = (C[f"{nm}M_{side}"][0:M_ROWS_PP].T
                             @ masterw[:, lo:hi]
                             + C[f"{nm}S_{side}"].T @ sq[:, lo:hi])
        return out

    l1a, b1, b2 = pack("L1A"), pack("L1BN"), pack("L1BX")
    m1 = b1 * l1a                                            # [96, W]
    m2 = b2 * l1a
    q01r = C["REP01RN"].T @ m1                               # [128, W]
    q01i = C["REP01IN"].T @ m2
    q2 = C["REP2RN"].T @ m1 + C["REP2IN"].T @ m2             # [128, W]

    def packc(nms):
        m = C[f"{nms}M_H"].shape[1]
        out = np.zeros((m, W), np.float32)
        for lo, hi, side in [(0, F, "H"), (F, W, "G")]:
            out[:, lo:hi] = (C[f"{nms}M_{side}"].T @ masterw[:, lo:hi]
                             + C[f"{nms}S_{side}"].T @ sq[:, lo:hi])
        return out

    c01r, c01i = packc("TABC01R"), packc("TABC01I")
    c2 = packc("TABC2")

    er01 = q01r * c01r - q01i * c01i                         # [128, W]
    ei01 = q01r * c01i + q01i * c01r
    er2 = q2[0:64] * c2[0:64] - q2[64:128] * c2[64:128]      # [64, W]
    ei2 = q2[0:64] * c2[64:128] + q2[64:128] * c2[0:64]

    def tmm(er, ei, rows):
        gr = er[rows, F:W]                                   # [64, P]
        gi = ei[rows, F:W]
        hr = er[rows, 0:F]                                   # [64, F]
        hi = ei[rows, 0:F]
        tr = gr.T @ hr - gi.T @ hi                           # [P, F]
        ti = gr.T @ hi + gi.T @ hr
        return tr, ti

    t0 = tmm(er01, ei01, slice(0, 64))
    t1 = tmm(er01, ei01, slice(64, 128))
    t2 = tmm(er2, ei2, slice(0, 64))

    p01r = t0[0] * t1[0] - t0[1] * t1[1]
    p01i = t0[0] * t1[1] + t0[1] * t1[0]
    perm_r = float(np.sum(p01r * t2[0]) - np.sum(p01i * t2[1]))
    perm_i = float(np.sum(p01r * t2[1]) + np.sum(p01i * t2[0]))
    pa2 = np.float32(perm_r) ** 2 + np.float32(perm_i) ** 2
    return EMU * SCALE2 * pa2 + DARK


# ---------------------------------------------------------------- device
FP16 = mybir.dt.float16

# CPKH: fp32 selection constants, column-packed [128, *]. M-parts read the
# master tile (96 rows), S-parts the squares tile (80), REP packs m1/m2 (96).
_CPKH_ORDER = ["REP01RN", "REP01IN", "REP2RN", "REP2IN",
               "L1AS_H", "L1AS_G", "L1BNS_H", "L1BNS_G", "L1BXS_H", "L1BXS_G",
               "TABC01RS_H", "TABC01RS_G", "TABC01IS_H", "TABC01IS_G",
               "TABC2S_H", "TABC2S_G"]


def _build_cpkh():
    # t48 combined tile [112, W]: r rows 0:48, i rows 64:112. REP lhsTs pick
    # from it: REP01R/REP01I [112, 128] (quad replication for groups 0,1 from
    # r / i rows), REP2RI [112, 128] (g2: cols 0:64 <- r rows, 64:128 <- i).
    C = dict(CONSTS)
    rep01 = C["REP01"]                     # [48, 128]
    r01r = np.zeros((112, 128), np.float32)
    r01r[0:48, :] = rep01
    r01i = np.zeros((112, 128), np.float32)
    r01i[64:112, :] = rep01
    C["REP01R"], C["REP01I"] = r01r, r01i
    r2 = np.zeros((112, 128), np.float32)
    r2[0:48, :] = C["REP2R"]
    r2[64:112, 0:128] = C["REP2I"]
    C["REP2RI"] = r2

    cols = {}
    blocks = []
    off = 0
    for nm in _CPKH_ORDER:
        arr = C[nm]
        a = np.zeros((128, arr.shape[1]), np.float32)
        a[0:arr.shape[0], :] = arr
        cols[nm] = (off, arr.shape[1], arr.shape[0])
        blocks.append(a)
        off += arr.shape[1]
    return np.concatenate(blocks, axis=1).astype(np.float32), cols


CPKH, CPKH_COLS = _build_cpkh()


def host_inputs(A_real_b: np.ndarray, A_imag_b: np.ndarray) -> dict:
    """Per-core input map for one batch element (pure reindexing of A)."""
    return {"LTPACK": build_ltpack2(A_real_b, A_imag_b),
            "MASKC": CONSTS["MASK"], "CPKH": CPKH}


def build_kernel(loop_iters=None):
    nc = bacc.Bacc("TRN2", target_bir_lowering=False, debug=False)
    tens = {}
    tens["LTPACK"] = nc.dram_tensor("LTPACK", [22, LT2_W], FP32R,
                                    kind="ExternalInput").ap()
    tens["MASKC"] = nc.dram_tensor("MASKC", [22, W], FP32R,
                                   kind="ExternalInput").ap()
    tens["CPKH"] = nc.dram_tensor("CPKH", [128, CPKH.shape[1]], FP32R,
                                  kind="ExternalInput").ap()
    tens["OUT"] = nc.dram_tensor("OUT", [1, 1], FP32, kind="ExternalOutput").ap()

    with tile.TileContext(nc) as tc:
        from contextlib import ExitStack
        ctx = ExitStack()
        pools = (ctx.enter_context(tc.tile_pool(name="pers", bufs=1)),
                 ctx.enter_context(tc.tile_pool(name="cm", bufs=2)),
                 ctx.enter_context(tc.tile_pool(name="psum", bufs=2,
                                                space="PSUM")))
        if loop_iters is None:
            tiles = _load(nc, pools, tens)
            _body(nc, tc, tens, pools, tiles)
        else:
            # timing build: reload + full body each iteration so the
            # loop-slope measurement reflects a complete dispatch
            with tc.For_i(0, loop_iters, 1):
                tiles = _load(nc, pools, tens)
                _body(nc, tc, tens, pools, tiles)
        ctx.close()
    nc.compile()
    return nc


def _load(nc, pools, tens):
    """Input DMAs: loop-invariant, issued once outside any timing loop."""
    pers, _cm, _psum = pools
    dma = nc.sync.dma_start
    cpk32 = pers.tile([22, W], FP32R, tag="cpk32")
    for lo, hi, _s in CHUNKS:
        dma(cpk32[:, lo:hi], tens["MASKC"][:, lo:hi])
    ltp = pers.tile([22, LT2_W], FP32R, tag="ltp")
    lt_l1 = LT2_OFF[("TABC01R", "H")]
    nc.scalar.dma_start(ltp[:, 0:160], tens["LTPACK"][:, 0:160])
    dma(ltp[:, 160:lt_l1], tens["LTPACK"][:, 160:lt_l1])
    dma(ltp[:, lt_l1:LT2_W], tens["LTPACK"][:, lt_l1:LT2_W])
    cpkh = pers.tile([128, CPKH.shape[1]], FP32R, tag="cpkh")
    nc.gpsimd.dma_start(cpkh[:], tens["CPKH"][:, :])
    return cpk32, ltp, cpkh


def _body(nc, tc, tens, pools, tiles):
    pers, cm, psum_pool = pools
    cpk32, ltp, cpkh = tiles
    dma = nc.sync.dma_start

    def csel(name, krows):
        off, width, _ = CPKH_COLS[name]
        return cpkh[0:krows, off:off + width]

    def emul(eng, out, a, b):
        eng.tensor_mul(out, a, b)

    def eaddsub(eng, out, a, b, op):
        if op == OP.add:
            eng.tensor_add(out, a, b)
        else:
            eng.tensor_sub(out, a, b)

    def cmul6(rows, w, i0, i1, outr, outi, dt=FP32, engs=None):
        """Complex multiply; engs = 6-tuple of engines for op distribution."""
        if engs is None:
            engs = (nc.vector,) * 6
        i0r, i0i = i0
        i1r, i1i = i1
        t1 = cm.tile([rows, w], dt, tag="cm_t1")
        t2 = cm.tile([rows, w], dt, tag="cm_t2")
        t3 = cm.tile([rows, w], dt, tag="cm_t3")
        t4 = cm.tile([rows, w], dt, tag="cm_t4")
        emul(engs[0], t1[:], i0r, i1r)
        emul(engs[1], t2[:], i0i, i1i)
        eaddsub(engs[2], outr, t1[:], t2[:], OP.subtract)
        emul(engs[3], t3[:], i0r, i1i)
        emul(engs[4], t4[:], i0i, i1r)
        eaddsub(engs[5], outi, t3[:], t4[:], OP.add)

    def R(ap):
        return ap.bitcast(FP32R)

    def sel_mm(const_name, rhs_tile, m, krows, rhs_rows, per_side=True,
               s_name=None, sq_tile=None):
        ps = psum_pool.tile([m, W], FP32, tag="ps")
        for lo, hi, side in CHUNKS:
            key = f"{const_name}_{side}" if per_side else const_name
            nc.tensor.matmul(ps[:, lo:hi], R(csel(key, krows)),
                             R(rhs_tile[0:rhs_rows, lo:hi]),
                             start=True, stop=(s_name is None))
            if s_name is not None:
                skey = f"{s_name}_{side}"
                nc.tensor.matmul(ps[:, lo:hi], R(csel(skey, SQROWS)),
                                 R(sq_tile[0:SQROWS, lo:hi]),
                                 start=False, stop=True)
        return ps

    def evac(ps, rows, w, tag, dt=FP32, eng=None):
        sb = pers.tile([rows, w], dt, tag=tag)
        if eng is nc.vector:
            eng.tensor_copy(sb[:], ps[0:rows, :])
        else:
            (eng or nc.scalar).copy(sb[:], ps[0:rows, :])
        return sb

    def evac3(ps, rows, tag, dt=FP32):
        """Per-chunk evac: each chunk lands as soon as its matmul is done."""
        sb = pers.tile([rows, W], dt, tag=tag)
        for lo, hi, _s in CHUNKS:
            nc.scalar.copy(sb[:, lo:hi], ps[0:rows, lo:hi])
        return sb


    # ---- square-input sums (folds in LTPACK cols 0:160), ACT Square evac;
    # the master row-sums are folded into the pack lhsT columns on the host
    ps_sq = psum_pool.tile([SQROWS, W], FP32, tag="ps")
    for lo, hi, side in CHUNKS:
        if side == "H":
            nc.tensor.matmul(ps_sq[:, lo:hi], ltp[0:22, 0:80],
                             cpk32[0:22, lo:hi], start=True, stop=True)
        else:
            nc.tensor.matmul(ps_sq[:, lo:hi], ltp[0:16, 80:160],
                             cpk32[0:16, lo:hi], start=True, stop=True)
    sqt = pers.tile([SQROWS, W], FP32R, tag="sqt")
    nc.scalar.activation(sqt[:], ps_sq[0:SQROWS, :], AF.Square)

    def fpack(name, m, s_name):
        ps = psum_pool.tile([m, W], FP32, tag="ps")
        oh, og = LT2_OFF[(name, "H")], LT2_OFF[(name, "G")]
        for lo, hi, side in CHUNKS:
            if side == "H":
                nc.tensor.matmul(ps[:, lo:hi], ltp[0:22, oh:oh + m],
                                 cpk32[0:22, lo:hi], start=True, stop=False)
            else:
                nc.tensor.matmul(ps[:, lo:hi], ltp[0:16, og:og + m],
                                 cpk32[0:16, lo:hi], start=True, stop=False)
            nc.tensor.matmul(ps[:, lo:hi], R(csel(f"{s_name}_{side}", SQROWS)),
                             R(sqt[0:SQROWS, lo:hi]), start=False, stop=True)
        return ps

    # ---- quad layer: raw-product tiles m1/m2 [96, W]; the +/- combine into
    # t48 quantities is folded into the REP*N constants downstream.
    ps_a = fpack("L1A", 96, "L1AS")
    a_sb = evac(ps_a, 96, W, "a_sb")
    ps_b1 = fpack("L1BN", 96, "L1BNS")
    ps_b2 = fpack("L1BX", 96, "L1BXS")
    # m1/m2 stay exact fp32; the q packs below run as plain fp32 matmuls
    m1 = pers.tile([96, W], FP32, tag="m1")
    m2 = pers.tile([96, W], FP32, tag="m2")
    nc.vector.tensor_mul(m1[:], ps_b1[0:96, :], a_sb[:])
    nc.vector.tensor_mul(m2[:], ps_b2[0:96, :], a_sb[:])

    # ---- L2 operand packs; TABC evac'd, q01 consumed from PSUM
    c01r = evac(fpack("TABC01R", 128, "TABC01RS"), 128, W, "c01r")
    c01i = evac(fpack("TABC01I", 128, "TABC01IS"), 128, W, "c01i")
    # ---- L2 raw products; q01 packs first (01 path is the critical chain)
    def qpack(cname, rhs):
        ps = psum_pool.tile([128, W], FP32, tag="ps")
        for lo, hi, _s in CHUNKS:
            nc.tensor.matmul(ps[:, lo:hi], csel(cname, 96).bitcast(FP32),
                             rhs[0:96, lo:hi], start=True, stop=True)
        return ps

    ps_q01r = qpack("REP01RN", m1)
    ps_q01i = qpack("REP01IN", m2)
    q01i = evac(ps_q01i, 128, W, "q01i")
    c2 = evac(fpack("TABC2", 128, "TABC2S"), 128, W, "c2", eng=nc.vector)
    t1a = cm.tile([128, W], FP32, tag="cm_t1")
    t2a = cm.tile([128, W], FP32, tag="cm_t2")
    t3a = cm.tile([128, W], FP32, tag="cm_t3")
    t4a = cm.tile([128, W], FP32, tag="cm_t4")
    emul(nc.vector, t1a[:], ps_q01r[:], c01r[:])
    emul(nc.gpsimd, t2a[:], q01i[:], c01i[:])
    emul(nc.vector, t3a[:], ps_q01r[:], c01i[:])
    emul(nc.gpsimd, t4a[:], q01i[:], c01r[:])
    er01 = pers.tile([128, W], FP32R, tag="er01")
    ei01 = pers.tile([128, W], FP32R, tag="ei01")
    eaddsub(nc.vector, er01[:], t1a[:], t2a[:], OP.subtract)
    eaddsub(nc.vector, ei01[:], t3a[:], t4a[:], OP.add)

    ps_q2 = psum_pool.tile([128, W], FP32, tag="ps")
    for lo, hi, _s in CHUNKS:
        nc.tensor.matmul(ps_q2[:, lo:hi], csel("REP2RN", 96).bitcast(FP32),
                         m1[0:96, lo:hi], start=True, stop=False)
        nc.tensor.matmul(ps_q2[:, lo:hi], csel("REP2IN", 96).bitcast(FP32),
                         m2[0:96, lo:hi], start=False, stop=True)
    q2 = evac(ps_q2, 128, W, "q2")
    t1b = cm.tile([64, W], FP32, tag="cm_t1")
    t2b = cm.tile([64, W], FP32, tag="cm_t2")
    t3b = cm.tile([64, W], FP32, tag="cm_t3")
    t4b = cm.tile([64, W], FP32, tag="cm_t4")
    emul(nc.gpsimd, t1b[:], q2[0:64, :], c2[0:64, :])
    emul(nc.gpsimd, t2b[:], q2[64:128, :], c2[64:128, :])
    emul(nc.vector, t3b[:], ps_q2[0:64, :], c2[64:128, :])
    emul(nc.vector, t4b[:], ps_q2[64:128, :], c2[0:64, :])
    er2 = pers.tile([64, W], FP32R, tag="er2")
    ei2 = pers.tile([64, W], FP32R, tag="ei2")
    eaddsub(nc.vector, er2[:], t1b[:], t2b[:], OP.subtract)
    eaddsub(nc.gpsimd, ei2[:], t3b[:], t4b[:], OP.add)

    # stage 4 on views of er/ei; only the negated G-side imag needs a tile
    zer = pers.tile([128, P], FP32, tag="zer")
    nc.gpsimd.memset(zer[:], 0.0)
    eineg = pers.tile([128, P], FP32R, tag="eineg")
    nc.gpsimd.tensor_sub(eineg[:], zer[:], ei01[:, F:W])
    eineg2 = pers.tile([64, P], FP32R, tag="eineg2")
    nc.gpsimd.tensor_sub(eineg2[:], zer[0:64, :], ei2[:, F:W])

    def tmm(g, which, tg="ps", bf=None):
        if g < 2:
            sl = slice(64 * g, 64 * g + 64)
            src_r, src_i, neg = er01, ei01, eineg
        else:
            sl = slice(0, 64)
            src_r, src_i, neg = er2, ei2, eineg2
        gr, gi, gin = src_r[sl, F:W], src_i[sl, F:W], neg[sl, :]
        ps = psum_pool.tile([P, F], FP32, tag=tg, bufs=bf)
        for c0 in range(0, F, 512):
            c1 = c0 + 512
            if which == "r":   # Tr = gr^T hr - gi^T hi
                nc.tensor.matmul(ps[:, c0:c1], R(gr), R(src_r[sl, c0:c1]),
                                 start=True, stop=False)
                nc.tensor.matmul(ps[:, c0:c1], R(gin), R(src_i[sl, c0:c1]),
                                 start=False, stop=True)
            else:              # Ti = gi^T hr + gr^T hi
                nc.tensor.matmul(ps[:, c0:c1], R(gi), R(src_r[sl, c0:c1]),
                                 start=True, stop=False)
                nc.tensor.matmul(ps[:, c0:c1], R(gr), R(src_i[sl, c0:c1]),
                                 start=False, stop=True)
        return ps

    # ---- T0 evac'd; T1 consumed from PSUM by the p01 multiplies; T2 last
    b0r = evac(tmm(0, "r", "t4", 1), P, F, "B0r")
    b0i = evac(tmm(0, "i"), P, F, "B0i")
    b1r = evac(tmm(1, "r"), P, F, "B1r")
    ps_t1i = tmm(1, "i", "t4", 1)

    t1p = cm.tile([P, F], FP32, tag="cm_t1")
    t2p = cm.tile([P, F], FP32, tag="cm_t2")
    t3p = cm.tile([P, F], FP32, tag="cm_t3")
    t4p = cm.tile([P, F], FP32, tag="cm_t4")
    nc.vector.tensor_mul(t2p[:], ps_t1i[:], b0i[:])
    nc.vector.tensor_mul(t4p[:], ps_t1i[:], b0r[:])
    emul(nc.gpsimd, t1p[:], b1r[:], b0r[:])
    emul(nc.gpsimd, t3p[:], b1r[:], b0i[:])
    p01r = pers.tile([P, F], FP32, tag="p01r")
    p01i = pers.tile([P, F], FP32, tag="p01i")
    nc.vector.tensor_sub(p01r[:], t1p[:], t2p[:])
    eaddsub(nc.gpsimd, p01i[:], t3p[:], t4p[:], OP.add)

    ps_t2r = tmm(2, "r")
    ps_t2i = tmm(2, "i")

    # t2s = T2r + T2i accumulated directly in PSUM: a free operand-sum for
    # the Karatsuba final reduce (8 matmuls on an otherwise idle PE)
    sl2 = slice(0, 64)
    gr2v, gi2v, gin2v = er2[sl2, F:W], ei2[sl2, F:W], eineg2[sl2, :]
    ps_t2s = psum_pool.tile([P, F], FP32, tag="t4", bufs=1)
    for c0 in range(0, F, 512):
        c1 = c0 + 512
        nc.tensor.matmul(ps_t2s[:, c0:c1], R(gr2v), R(er2[sl2, c0:c1]),
                         start=True, stop=False)
        nc.tensor.matmul(ps_t2s[:, c0:c1], R(gin2v), R(ei2[sl2, c0:c1]),
                         start=False, stop=False)
        nc.tensor.matmul(ps_t2s[:, c0:c1], R(gi2v), R(er2[sl2, c0:c1]),
                         start=False, stop=False)
        nc.tensor.matmul(ps_t2s[:, c0:c1], R(gr2v), R(ei2[sl2, c0:c1]),
                         start=False, stop=True)

    # Karatsuba reduce: m1 = sum p01r*t2r, m2 = sum p01i*t2i,
    # m3 = sum (p01r+p01i)*(t2r+t2i); perm_r = m1-m2, perm_i = m3-m1-m2
    s_p = pers.tile([P, F], FP32, tag="s_p")
    nc.gpsimd.tensor_add(s_p[:], p01r[:], p01i[:])
    scr1 = pers.tile([P, F], FP32, tag="scr1")
    scr2 = pers.tile([P, F], FP32, tag="scr2")
    scr3 = pers.tile([P, F], FP32, tag="scr3")
    acc = pers.tile([P, 4], FP32, tag="acc")
    nc.vector.scalar_tensor_tensor(
        out=scr1[:], in0=p01r[:], scalar=1.0, in1=ps_t2r[:],
        op0=OP.mult, op1=OP.mult, accum_out=acc[:, 0:1])
    nc.vector.scalar_tensor_tensor(
        out=scr2[:], in0=p01i[:], scalar=1.0, in1=ps_t2i[:],
        op0=OP.mult, op1=OP.mult, accum_out=acc[:, 1:2])
    nc.vector.scalar_tensor_tensor(
        out=scr3[:], in0=s_p[:], scalar=1.0, in1=ps_t2s[:],
        op0=OP.mult, op1=OP.mult, accum_out=acc[:, 2:3])

    ones = pers.tile([P, 1], FP32, tag="ones")
    nc.gpsimd.memset(ones[:], 1.0)
    dark = pers.tile([1, 1], FP32, tag="dark")
    nc.gpsimd.memset(dark[:], float(DARK))
    accs = pers.tile([P, 2], FP32, tag="accs")
    nc.vector.tensor_sub(accs[:, 0:1], acc[:, 0:1], acc[:, 1:2])
    nc.vector.tensor_sub(acc[:, 3:4], acc[:, 2:3], acc[:, 0:1])
    nc.vector.tensor_sub(accs[:, 1:2], acc[:, 3:4], acc[:, 1:2])
    red = psum_pool.tile([1, 2], FP32, tag="t4", bufs=1)
    nc.tensor.matmul(red[:], ones[:], accs[:], start=True, stop=True)

    p2 = pers.tile([1, 2], FP32, tag="p2")
    nc.scalar.activation(p2[:], red[:], AF.Square)
    pa2 = pers.tile([1, 1], FP32, tag="pa2")
    nc.vector.reduce_sum(pa2[:], p2[:], axis=mybir.AxisListType.X)
    res = pers.tile([1, 1], FP32, tag="res")
    nc.vector.scalar_tensor_tensor(
        out=res[:], in0=pa2[:], scalar=float(EMU * SCALE2), in1=dark[:],
        op0=OP.mult, op1=OP.add)
    dma(tens["OUT"][:, :], res[:])


def kernel(A_real: np.ndarray, A_imag: np.ndarray) -> np.ndarray:
    Bn = A_real.shape[0]
    assert Bn == 8 and A_real.shape == (Bn, N, N)
    if "nc" not in _CACHE:
        _CACHE["nc"] = build_kernel()
    nc = _CACHE["nc"]
    in_maps = [host_inputs(np.asarray(A_real[b], dtype=np.float32),
                           np.asarray(A_imag[b], dtype=np.float32))
               for b in range(Bn)]
    res = run_bass_kernel_spmd(nc, in_maps, list(range(Bn)))
    out = np.array([res.results[b]["OUT"].reshape(-1)[0] for b in range(Bn)],
                   dtype=np.float32)
    return out


def ref_prob(ar, ai):
    n = N
    idx = np.arange(2 ** n, dtype=np.uint32)
    mask = ((idx[:, None] >> np.arange(n, dtype=np.uint32)[None, :]) & 1
            ).astype(np.float32)
    k = mask.sum(axis=1)
    sign = np.where((n - k) % 2 == 0, 1.0, -1.0).astype(np.complex64)
    rs = (mask @ ar.T.astype(np.float32)
          + 1j * (mask @ ai.T.astype(np.float32))).astype(np.complex64)
    prod = rs[:, 0].copy()
    for i in range(1, n):
        prod = prod * rs[:, i]
    perm = np.sum(prod * sign)
    pa2 = np.float32(perm.real) ** 2 + np.float32(perm.imag) ** 2
    return EMU * pa2 + DARK


if __name__ == "__main__":
    rng = np.random.default_rng(0)
    for t in range(3):
        Ar = (rng.standard_normal((N, N)) / np.sqrt(N)).astype(np.float32)
        Ai = (rng.standard_normal((N, N)) / np.sqrt(N)).astype(np.float32)
        got = emulate(Ar, Ai)
        want = ref_prob(Ar, Ai)
        print(f"emu={got:.6e} ref={want:.6e} rel={abs(got-want)/abs(want):.2e}")

